# revision 27
# baseline (speedup 1.0000x reference)
"""Multi-head attention (B=4, S=2048, D=512, H=8, Dh=64) on 8 trn2 NeuronCores.

Sharding: core c = b*2 + hg handles batch b and head-group hg (4 heads).
Per core: project Q^T/K^T (head-dim on partitions) and V (natural layout),
compute scores^T = K Q^T per head with the PE array row-tiled into two
64-contraction halves (head pairs), exp on ScalarE with the 1/sqrt(dh)
scale folded in, P^T@V via a [keys,65] stationary V with a ones column so
softmax row-sums fall out of the same matmul, normalize with
reciprocal+broadcast, then a row-parallel out-projection. The two
head-group partial outputs are summed on host (plus bv@Wo + bo).
"""
import numpy as np

import concourse.bass as bass
import concourse.mybir as mybir
import concourse.tile as tile
from concourse.bass_utils import run_bass_kernel_spmd
from concourse.masks import make_identity

F32 = mybir.dt.float32
F32R = mybir.dt.float32r

B, S, D_IN, H, D_HEAD = 4, 2048, 512, 8, 64
HG = 2                      # head groups (tensor-parallel shards)
H_LOC = H // HG             # 4 heads per core
DO = H_LOC * D_HEAD         # 256 projected dims per core
N_CORES = B * HG
P = 128
ST = S // P                 # 16 s-tiles
KC = D_IN // P              # 4 contraction chunks
QB = 4                      # q blocks
QBS = S // QB               # 512 q block size
NPAIR = H_LOC // 2          # 2 head pairs

# ---------------------------------------------------------------------------
# walrus in this container rejects >1 sync-wait per instruction: split the
# extras onto single-wait NOPs inserted before the instruction (same engine).
_ENGINES_WITH_NOP = {
    mybir.EngineType.PE,
    mybir.EngineType.Activation,
    mybir.EngineType.DVE,
    mybir.EngineType.Pool,
    mybir.EngineType.SP,
}


def _split_multi_waits(nc, max_waits=1):
    cnt = 0
    for fn in nc.m.functions:
        for blk in fn.blocks:
            out = []
            changed = False
            for inst in blk.instructions:
                si = getattr(inst, "sync_info", None)
                waits = list(si.on_wait) if si is not None else []
                if len(waits) > max_waits and inst.engine in _ENGINES_WITH_NOP:
                    changed = True
                    for w in waits[:-max_waits]:
                        cnt += 1
                        out.append(
                            mybir.InstNoOp(
                                name=f"I-wsplit-{cnt}",
                                engine=inst.engine,
                                ins=[],
                                outs=[],
                                sync_info=mybir.SyncInfo(on_wait=[w], on_update=[]),
                            )
                        )
                    inst.sync_info = mybir.SyncInfo(
                        on_wait=waits[-max_waits:], on_update=list(si.on_update)
                    )
                out.append(inst)
            if changed:
                blk.instructions = out


# ---------------------------------------------------------------------------


def build_program(loop_iters=None, parts=("scores", "exp", "av", "norm", "outproj")):
    nc = bass.Bass()

    xq = nc.declare_dram_parameter("xq", [S, D_IN], F32, isOutput=False)
    xk = nc.declare_dram_parameter("xk", [S, D_IN], F32, isOutput=False)
    xv = nc.declare_dram_parameter("xv", [S, D_IN], F32, isOutput=False)
    wq = nc.declare_dram_parameter("wq", [D_IN, DO], F32, isOutput=False)
    wk = nc.declare_dram_parameter("wk", [D_IN, DO], F32, isOutput=False)
    wv = nc.declare_dram_parameter("wv", [D_IN, DO], F32, isOutput=False)
    wo = nc.declare_dram_parameter("wo", [DO, D_HEAD], F32, isOutput=False)
    bqp = nc.declare_dram_parameter("bq", [DO], F32, isOutput=False)
    bkp = nc.declare_dram_parameter("bk", [DO], F32, isOutput=False)
    out = nc.declare_dram_parameter("out", [S, D_HEAD], F32, isOutput=True)

    with tile.TileContext(nc) as tc:
        with (
            tc.tile_pool(name="cst", bufs=1) as cst,
            tc.tile_pool(name="stage", bufs=4) as stage,
            tc.tile_pool(name="xT", bufs=2) as xT_pool,
            tc.tile_pool(name="proj", bufs=1) as proj_pool,
            tc.tile_pool(name="attnp", bufs=4) as attn_pool,
            tc.tile_pool(name="expp", bufs=6) as exp_pool,
            tc.tile_pool(name="small", bufs=4) as small,
            tc.tile_pool(name="outst", bufs=3) as outst,
            tc.tile_pool(name="big_ps", bufs=2, space="PSUM") as big_ps,
            tc.tile_pool(name="ps1", bufs=4, space="PSUM") as ps1,
        ):
            ident = cst.tile([P, P], F32)
            make_identity(nc, ident[:])

            # biases as [128, 2] per-partition columns
            bq_sb = cst.tile([P, 2], F32)
            bk_sb = cst.tile([P, 2], F32)
            for mc in range(2):
                nc.sync.dma_start(bq_sb[:, mc : mc + 1], bqp[mc * P : (mc + 1) * P, None])
                nc.sync.dma_start(bk_sb[:, mc : mc + 1], bkp[mc * P : (mc + 1) * P, None])

            # weights: dma fp32, round to fp32r
            w_r = {}
            for name, ap in (("wq", wq), ("wk", wk), ("wv", wv)):
                wt = stage.tile([P, KC, DO], F32, tag="wstage")
                nc.sync.dma_start(wt[:], ap.rearrange("(c p) o -> p c o", p=P))
                wr = cst.tile([P, KC, DO], F32R, name=f"{name}_r")
                nc.vector.tensor_copy(out=wr[:], in_=wt[:])
                w_r[name] = wr
            wo_sb = cst.tile([P, 2, D_HEAD], F32)
            nc.sync.dma_start(wo_sb[:], wo.rearrange("(c p) o -> p c o", p=P))

            from contextlib import ExitStack as _ES
            _loop = _ES()
            if loop_iters is not None:
                _loop.enter_context(tc.For_i(0, loop_iters, 1))

            # ------------------------------------------------------------------
            # Phase A: transpose inputs on PE in half-tensors; project each half
            # as soon as it lands so proj overlaps the next half's transposes.
            SH = S // 2
            dma_eng = {"xq": nc.sync, "xk": nc.scalar, "xv": nc.sync}

            qtp = [proj_pool.tile([P, S], F32R, name=f"QTp{i}") for i in range(2)]
            ktp = [proj_pool.tile([P, S], F32R, name=f"KTp{i}") for i in range(2)]
            v_sb = [
                proj_pool.tile([P, ST, 2, D_HEAD + 1], F32R, name=f"Vsb{i}")
                for i in range(2)
            ]
            onescol = cst.tile([P, 1], F32)
            nc.vector.memset(onescol[:], 1.0)
            for i in range(2):
                nc.vector.tensor_copy(
                    out=v_sb[i][:, :, :, D_HEAD : D_HEAD + 1],
                    in_=onescol[:, None, None, :].to_broadcast((P, ST, 2, 1)),
                )

            def transpose_half(name, ap, h, xt):
                # s-tiles [h*8, h*8+8) of `ap` -> xt [P, KC, SH] (f32r)
                for g in range(2):
                    x_sb = stage.tile(
                        [P, 4, D_IN], F32, tag="xstage", name=f"xs{name}{h}{g}"
                    )
                    base = (h * 8 + g * 4) * P
                    dma_eng[name].dma_start(
                        x_sb[:],
                        ap[base : base + 4 * P, :].rearrange("(t p) d -> p t d", p=P),
                    )
                    for t in range(4):
                        lst = g * 4 + t  # local s-tile within the half
                        for c in range(KC):
                            tp = ps1.tile([P, P], F32, tag="ps1")
                            nc.tensor.transpose(
                                tp[:], x_sb[:, t, c * P : (c + 1) * P], ident[:]
                            )
                            if c % 2 == 0:
                                nc.vector.tensor_copy(
                                    out=xt[:, c, lst * P : (lst + 1) * P], in_=tp[:]
                                )
                            else:
                                nc.scalar.copy(
                                    out=xt[:, c, lst * P : (lst + 1) * P], in_=tp[:]
                                )

            for name, ap, wname, dsts, bias in (
                ("xq", xq, "wq", qtp, bq_sb),
                ("xk", xk, "wk", ktp, bk_sb),
            ):
                for h in range(2):
                    xt = xT_pool.tile([P, KC, SH], F32R, tag="xT", name=f"{name}T{h}")
                    transpose_half(name, ap, h, xt)
                    for mc in range(2):
                        for lqc in range(2):  # 512-chunks within the half
                            qc = h * 2 + lqc
                            ps = big_ps.tile([P, 2, QBS], F32, tag="big")
                            for kc in range(KC):
                                nc.tensor.matmul(
                                    ps[:, 0, :],
                                    w_r[wname][:, kc, mc * P : (mc + 1) * P],
                                    xt[:, kc, lqc * QBS : (lqc + 1) * QBS],
                                    start=(kc == 0),
                                    stop=(kc == KC - 1),
                                )
                            nc.vector.tensor_scalar(
                                out=dsts[mc][:, qc * QBS : (qc + 1) * QBS],
                                in0=ps[:, 0, :],
                                scalar1=bias[:, mc : mc + 1],
                                scalar2=None,
                                op0=mybir.AluOpType.add,
                            )

            for h in range(2):
                xt = xT_pool.tile([P, KC, SH], F32R, tag="xT", name=f"xvT{h}")
                transpose_half("xv", xv, h, xt)
                for lst in range(8):
                    st = h * 8 + lst
                    ps = big_ps.tile([P, 2, QBS], F32, tag="big")
                    for kc in range(KC):
                        nc.tensor.matmul(
                            ps[:, 0, :DO],
                            xt[:, kc, lst * P : (lst + 1) * P],
                            w_r["wv"][:, kc, :],
                            start=(kc == 0),
                            stop=(kc == KC - 1),
                        )
                    for i in range(2):
                        nc.vector.tensor_copy(
                            out=v_sb[i][:, st, :, 0:D_HEAD],
                            in_=ps[
                                :, 0, i * 2 * D_HEAD : (i + 1) * 2 * D_HEAD
                            ].rearrange("p (h d) -> p h d", h=2),
                        )

            # ------------------------------------------------------------------
            # Phase B: attention (p outer), out-projection afterwards.
            attn = proj_pool.tile([P, 2, S], F32R, name="attnT")
            for p in range(NPAIR):
                for qb in range(QB):
                    oT = [
                        ps1.tile([D_HEAD + 1, QBS], F32, tag="ps1", name=f"oT{i}")
                        for i in range(2)
                    ]
                    for kt in range(ST):
                        if "scores" not in parts:
                            break
                        sc = big_ps.tile([P, 2, QBS], F32, tag="big")
                        for f in range(2):
                            nc.tensor.matmul(
                                sc[:, f, :],
                                ktp[p][f * 64 : (f + 1) * 64, kt * P : (kt + 1) * P],
                                qtp[p][f * 64 : (f + 1) * 64, qb * QBS : (qb + 1) * QBS],
                                start=True,
                                stop=True,
                            )
                        if "exp" not in parts:
                            continue
                        ex = exp_pool.tile([P, 2, QBS], F32R, tag="exp")
                        nc.scalar.activation(
                            ex[:], sc[:],
                            mybir.ActivationFunctionType.Exp,
                            scale=float(1.0 / np.sqrt(D_HEAD)),
                        )
                        if "av" not in parts:
                            continue
                        for f in range(2):
                            nc.tensor.matmul(
                                oT[f][:],
                                v_sb[p][:, kt, f, :],
                                ex[:, f, :],
                                start=(kt == 0),
                                stop=(kt == ST - 1),
                            )
                    # normalize: rows 0..63 / row 64
                    for f in range(2):
                        if "norm" not in parts or "av" not in parts:
                            break
                        rec = small.tile([1, QBS], F32, tag="rec")
                        nc.vector.reciprocal(rec[:], oT[f][D_HEAD : D_HEAD + 1, :])
                        rb = small.tile([D_HEAD, QBS], F32, tag="rb")
                        nc.sync.dma_start(
                            rb[:], rec[:, None, :].to_broadcast((1, D_HEAD, QBS))
                        )
                        nc.vector.tensor_tensor(
                            out=attn[f * 64 : (f + 1) * 64, p, qb * QBS : (qb + 1) * QBS],
                            in0=oT[f][0:D_HEAD, :],
                            in1=rb[:],
                            op=mybir.AluOpType.mult,
                        )

            # out projection at the end
            for qt in range(ST):
                if "outproj" not in parts:
                    o_sb = outst.tile([P, D_HEAD], F32, tag="ost", name="o_dummy")
                    nc.vector.tensor_copy(
                        out=o_sb[:],
                        in_=qtp[0][:, qt * P : qt * P + D_HEAD].bitcast(F32),
                    )
                    nc.sync.dma_start(out[qt * P : (qt + 1) * P, :], o_sb[:])
                    continue
                ps = ps1.tile([P, D_HEAD], F32, tag="ps1")
                for kc in range(2):
                    nc.tensor.matmul(
                        ps[:],
                        attn[:, kc, qt * P : (qt + 1) * P].bitcast(F32),
                        wo_sb[:, kc, :],
                        start=(kc == 0),
                        stop=(kc == 1),
                    )
                o_sb = outst.tile([P, D_HEAD], F32, tag="ost")
                nc.vector.tensor_copy(out=o_sb[:], in_=ps[:])
                nc.sync.dma_start(out[qt * P : (qt + 1) * P, :], o_sb[:])

            _loop.close()

    _split_multi_waits(nc)
    return nc


class _Runner:
    """Compile once; keep a jitted shard_map executable around.

    Mirrors bass2jax.run_bass_via_pjrt's multi-core path, but exposes the
    jitted fn + device-resident inputs so repeated timed executions don't
    pay re-transfer or re-trace costs.
    """

    def __init__(self, nc=None):
        import jax
        from jax.experimental.shard_map import shard_map
        from jax.sharding import Mesh, NamedSharding, PartitionSpec
        from concourse import bass2jax

        bass2jax.install_neuronx_cc_hook()
        if nc is None:
            nc = build_program()
        self.nc = nc
        self.jax = jax

        partition_name = (
            nc.partition_id_tensor.name if nc.partition_id_tensor else None
        )
        in_names, out_names, out_avals, zero_outs = [], [], [], []
        for alloc in nc.m.functions[0].allocations:
            if not isinstance(alloc, mybir.MemoryLocationSet):
                continue
            name = alloc.memorylocations[0].name
            if alloc.kind == "ExternalInput":
                if name != partition_name:
                    in_names.append(name)
            elif alloc.kind == "ExternalOutput":
                out_names.append(name)
                shape = tuple(alloc.tensor_shape)
                dtype = mybir.dt.np(alloc.dtype)
                out_avals.append(jax.core.ShapedArray(shape, dtype))
                zero_outs.append(np.zeros(shape, dtype))
        self.in_names = list(in_names)
        self.out_names = out_names
        self.out_avals = out_avals
        self.zero_outs = zero_outs
        n_params = len(in_names)
        n_outs = len(out_avals)
        all_in_names = in_names + out_names
        if partition_name is not None:
            all_in_names.append(partition_name)
        donate = tuple(range(n_params, n_params + n_outs))

        def _body(*args):
            operands = list(args)
            if partition_name is not None:
                operands.append(bass2jax.partition_id_tensor())
            outs = bass2jax._bass_exec_p.bind(
                *operands,
                out_avals=tuple(out_avals),
                in_names=tuple(all_in_names),
                out_names=tuple(out_names),
                lowering_input_output_aliases=(),
                sim_require_finite=True,
                sim_require_nnan=True,
                nc=nc,
            )
            return tuple(outs)

        devices = jax.devices()[:N_CORES]
        mesh = Mesh(np.asarray(devices), ("core",))
        self.mesh = mesh
        self.sharding = NamedSharding(mesh, PartitionSpec("core"))
        in_specs = (PartitionSpec("core"),) * (n_params + n_outs)
        out_specs = (PartitionSpec("core"),) * len(out_names)
        self.fn = jax.jit(
            shard_map(
                _body, mesh=mesh, in_specs=in_specs,
                out_specs=out_specs, check_rep=False,
            ),
            donate_argnums=donate,
            keep_unused=True,
        )

    def put_inputs(self, in_maps):
        concat = [
            np.concatenate([np.asarray(in_maps[c][n]) for c in range(N_CORES)], axis=0)
            for n in self.in_names
        ]
        return [self.jax.device_put(a, self.sharding) for a in concat]

    def make_zeros(self):
        return [
            self.jax.device_put(
                np.zeros((N_CORES * z.shape[0], *z.shape[1:]), z.dtype), self.sharding
            )
            for z in self.zero_outs
        ]

    def run(self, in_dev):
        out_arrs = self.fn(*in_dev, *self.make_zeros())
        return [
            {
                n: np.asarray(out_arrs[i]).reshape(N_CORES, *self.out_avals[i].shape)[c]
                for i, n in enumerate(self.out_names)
            }
            for c in range(N_CORES)
        ]


_RUNNER = None


def _get_runner():
    global _RUNNER
    if _RUNNER is None:
        _RUNNER = _Runner()
    return _RUNNER


def _make_in_maps(query, key, value, Wq, Wk, Wv, Wo, bq, bk):
    in_maps = []
    for c in range(N_CORES):
        b, hg = divmod(c, HG)
        sl = slice(hg * DO, (hg + 1) * DO)
        in_maps.append(
            {
                "xq": query[b],
                "xk": key[b],
                "xv": value[b],
                "wq": np.ascontiguousarray(Wq[:, sl]),
                "wk": np.ascontiguousarray(Wk[:, sl]),
                "wv": np.ascontiguousarray(Wv[:, sl]),
                "wo": np.ascontiguousarray(Wo[sl, :]),
                "bq": np.ascontiguousarray(bq[sl]),
                "bk": np.ascontiguousarray(bk[sl]),
            }
        )
    return in_maps


def kernel(query, key, value, Wq, bq, Wk, bk, Wv, bv, Wo, bo):
    query = np.ascontiguousarray(np.asarray(query, dtype=np.float32))
    key = np.ascontiguousarray(np.asarray(key, dtype=np.float32))
    value = np.ascontiguousarray(np.asarray(value, dtype=np.float32))
    Wq = np.asarray(Wq, dtype=np.float32)
    Wk = np.asarray(Wk, dtype=np.float32)
    Wv = np.asarray(Wv, dtype=np.float32)
    Wo = np.asarray(Wo, dtype=np.float32)
    bq = np.asarray(bq, dtype=np.float32)
    bk = np.asarray(bk, dtype=np.float32)
    bv = np.asarray(bv, dtype=np.float32)
    bo = np.asarray(bo, dtype=np.float32)

    r = _get_runner()
    in_dev = r.put_inputs(_make_in_maps(query, key, value, Wq, Wk, Wv, Wo, bq, bk))
    results = r.run(in_dev)

    out = np.zeros((B, S, D_HEAD), dtype=np.float32)
    for c in range(N_CORES):
        b = c // HG
        out[b] += results[c]["out"]
    out += bv @ Wo + bo
    return out


def bench(query, key, value, Wq, bq, Wk, bk, Wv, bv, Wo, bo, iters=20):
    """Steady-state per-iteration wall time of the device execution."""
    import time

    r = _get_runner()
    in_dev = r.put_inputs(
        _make_in_maps(
            np.asarray(query, np.float32), np.asarray(key, np.float32),
            np.asarray(value, np.float32), np.asarray(Wq, np.float32),
            np.asarray(Wk, np.float32), np.asarray(Wv, np.float32),
            np.asarray(Wo, np.float32), np.asarray(bq, np.float32),
            np.asarray(bk, np.float32),
        )
    )
    # warm up
    outs = r.fn(*in_dev, *r.make_zeros())
    self_jax = r.jax
    self_jax.block_until_ready(outs)
    # pre-stage zero buffers for every iteration (donated each call)
    zeros = [r.make_zeros() for _ in range(iters)]
    t0 = time.monotonic()
    last = None
    for i in range(iters):
        last = r.fn(*in_dev, *zeros[i])
    self_jax.block_until_ready(last)
    t1 = time.monotonic()
    return (t1 - t0) / iters
